# revision 32
# baseline (speedup 1.0000x reference)
"""ColBERT pairwise + in-batch negative CE loss on 8 Trainium2 NeuronCores.

Problem shapes (hardcoded): B=64, N=32, S=256, D=128, fp32.

reference:
    pos_scores[b]  = sum_n max_s  q[b,n,:] . d[b,s,:]
    neg_scores[b]  = sum_n max_s  q[b,n,:] . neg[b,s,:]
    scores[b,c]    = sum_n max_s  q[b,n,:] . d[c,s,:]
    loss = (mean softplus(neg_scores - pos_scores)
            + mean softplus(max_offdiag_c scores[b,c] - scores[b,b])) / 2

Sharding: in-batch scores sharded over the doc dim c (8 docs per core; every
core sees all 64*32 query rows).  The pairwise-neg term is data-parallel over
b.  All operands are host-pre-transposed to d-major so the contraction dim
d=128 maps onto the PE partition dim with zero device transposes.

The kernel's true bottleneck is the PSUM drain: every score element must
leave PSUM through the vector OR scalar engine at 1 elem/cycle/lane
(HW-measured; dual-PSUM-operand reads are illegal, tensor_reduce has no 2x
mode even for 16-bit, gpsimd has no max ops and no PSUM port, DMA cannot
read PSUM).  The 16 score units (each [128 q, 8 docs x 256 s] fp32 in PSUM,
double-buffered) are therefore split between the two engines so both drain
concurrently:

- D-units {0, 4, 8, 15}: vector engine reduce_max directly from PSUM
  (~2.3us per unit).  Unit 0's reduce is split in halves emitted between
  its matmuls so the first reduce fires after 2 of 4 MMs (shorter ramp);
  unit 15 is direct so the tail ends with a cheap reduce, not a tree.
- A-units (the other 12): scalar engine copies PSUM -> SBUF f16 (~2.0us,
  1 elem/cycle), and the vector engine finishes with a tensor_max halving
  tree, which runs at 2 results/cycle in 16-bit (2x_1p, consuming 4
  elem/cycle) and is batched over 3 units to amortize per-op overhead
  (~1.4us per unit).  maxall columns are assigned in drain order and
  remapped on the host.

Per core the device produces a (4, 130) fp32 tile:
  cols 0..127:  col 8*blk+c, row j -> sum_n max_s (q[. ] . d_local[c])
                (blk = drain-order block of query chunk m; host remaps)
  cols 128/129: col 128+g, row j  ->  neg_scores for local b = 4g+j
The host assembles the full (64, 64) scores matrix + the 64 neg pairwise
scores and applies the trivial softplus/mean epilogue (128 scalars).
"""

import sys

import numpy as np


def _ensure_path():
    try:
        import concourse  # noqa: F401
    except ImportError:
        sys.path.insert(0, "/opt/trn_rl_repo")


_ensure_path()

import concourse.bacc as bacc  # noqa: E402
import concourse.mybir as mybir  # noqa: E402
from concourse.bass_utils import run_bass_kernel_spmd  # noqa: E402
from concourse.tile import TileContext  # noqa: E402

B, N, S, D = 64, 32, 256, 128
NC = 8
CL = B // NC  # docs / queries per core (8)
BN = B * N  # 2048 query rows
DCOLS = CL * S  # 2048 doc columns per core
NEG_INF_DIAG = 1000000.0

F32 = mybir.dt.float32
F16 = mybir.dt.float16
MMDT = mybir.dt.float16  # dtype used by the matmul operands

_CACHE = {}


def _install_ntff_shim():
    """Best-effort: register the axon NTFF profile hook so BASS_TRACE=1
    produces hardware profiles.  Safe no-op when unavailable."""
    try:
        import types

        import antenv

        if "antenv.axon_hooks" in sys.modules:
            return
        import trn_agent_boot.trn_boot as tb

        mod = types.ModuleType("antenv.axon_hooks")
        _hook = [None]
        mod.set_axon_ntff_profile_hook = lambda h: _hook.__setitem__(0, h)
        mod.get_axon_ntff_profile_hook = lambda: _hook[0]
        sys.modules["antenv.axon_hooks"] = mod
        antenv.axon_hooks = mod
        mod.set_axon_ntff_profile_hook(
            tb._ntff_profile_via_ctypes("/opt/axon/libaxon_pjrt.so")
        )
    except Exception:
        pass


def _build():
    nc = bacc.Bacc("TRN2", target_bir_lowering=False, debug=False, num_devices=NC)
    qT = nc.dram_tensor("qT", [D, BN], MMDT, kind="ExternalInput")
    dT = nc.dram_tensor("dT", [D, DCOLS], MMDT, kind="ExternalInput")
    nT = nc.dram_tensor("nT", [D, DCOLS], MMDT, kind="ExternalInput")
    qp = nc.dram_tensor("qp", [D, CL * N], MMDT, kind="ExternalInput")
    ones = nc.dram_tensor("ones", [D, 4], F16, kind="ExternalInput")
    out_d = nc.dram_tensor("out", [4, 130], F32, kind="ExternalOutput")

    X = mybir.AxisListType.X

    with TileContext(nc) as tc:
        with (
            tc.tile_pool(name="sb", bufs=1) as sb,
            tc.tile_pool(name="ar", bufs=3) as arp,
            tc.tile_pool(name="tr", bufs=2) as trp,
            tc.tile_pool(name="ps", bufs=2, space="PSUM") as ps,
        ):
            qs = sb.tile([D, BN], MMDT, tag="qs")
            ds = sb.tile([D, DCOLS], MMDT, tag="ds")
            ns = sb.tile([D, DCOLS], MMDT, tag="ns")
            qps = sb.tile([D, CL * N], MMDT, tag="qps")
            onesb = sb.tile([D, 4], F16, tag="ones")
            maxall = sb.tile([128, 130], F16, tag="maxall")
            outsb = sb.tile([4, 130], F32, tag="outsb")

            # Input DMAs spread over four hardware queues (each engine can
            # issue DMAs) so ds+first q chunk land as early as possible.
            nc.sync.dma_start(out=qs[:, 0:128], in_=qT[:, 0:128])
            for p8 in range(8):
                eng = nc.sync if p8 % 2 == 0 else nc.scalar
                sl = slice(256 * p8, 256 * (p8 + 1))
                eng.dma_start(out=ds[:, sl], in_=dT[:, sl])
            nc.sync.dma_start(out=qs[:, 128:1024], in_=qT[:, 128:1024])
            nc.sync.dma_start(out=qs[:, 1024:2048], in_=qT[:, 1024:2048])
            for p, eng in zip(range(4), (nc.sync, nc.sync, nc.sync, nc.sync)):
                sl = slice(512 * p, 512 * (p + 1))
                eng.dma_start(out=ns[:, sl], in_=nT[:, sl])
            nc.sync.dma_start(out=qps[:, :], in_=qp[:, :])
            nc.sync.dma_start(out=onesb[:, :], in_=ones[:, :])

            # In-batch term: query chunk m (128 rows) x all 2048 local doc
            # cols -> PSUM [128, 2048].  D units {0,4,8,15} are reduced
            # directly by the vector engine (maxall col blocks 12..15); the
            # other 12 are drained by the scalar engine into an f16 arena
            # and max-tree'd by the vector engine in batches of 3 (col
            # blocks 0..11 in batch order; host remaps).  The pairwise term
            # is emitted mid-stream so its reduce stays off the tail, and
            # unit 15 is emitted before units 12-14 so the final tail is a
            # tree only.
            # Pre-warm the PE's HAM clock gate with dummy matmuls on
            # memset data (no DMA dependency) so the real matmuls run at
            # 2.4GHz from the start instead of paying ~3.4us of 1.2GHz ramp.
            wa = sb.tile([D, 128], F16, tag="wa")
            wb = sb.tile([D, 512], F16, tag="wb")
            nc.gpsimd.memset(wa[:, :], 0.0)
            nc.gpsimd.memset(wb[:, :], 0.0)
            wt = ps.tile([128, 2048], F32, tag="chunk", name="warm")
            for w in range(4):
                nc.tensor.matmul(
                    wt[:, 512 * (w % 4) : 512 * (w % 4 + 1)],
                    wa[:, :],
                    wb[:, :],
                    start=True,
                    stop=True,
                )

            DSET = {0, 8, 15}
            # A-unit batches for the f16 max-trees; the last batch is small
            # so the tail tree after the final ACT copy is short.
            BATCHES = [[1, 2, 3], [4, 5, 6, 7], [9, 10, 11, 12], [13, 14]]
            bat_of = {}
            for _bi, _bb in enumerate(BATCHES):
                for _sl, _mm in enumerate(_bb):
                    bat_of[_mm] = (_bi, _sl)
            colofs = [0]
            for _bb in BATCHES:
                colofs.append(colofs[-1] + 8 * len(_bb))
            state = {"arena": None, "nd": 0}

            def emit_unit(m):
                t = ps.tile([128, 2048], F32, tag="chunk", name=f"u{m}")
                halves = (m == 0)
                for u in range(4):
                    nc.tensor.matmul(
                        t[:, 512 * u : 512 * (u + 1)],
                        qs[:, 128 * m : 128 * (m + 1)],
                        ds[:, 512 * u : 512 * (u + 1)],
                        start=True,
                        stop=True,
                    )
                    if halves and u == 1:
                        nc.vector.reduce_max(
                            maxall[:, 104:108],
                            t[:, 0:1024].rearrange("p (g s) -> p g s", s=S),
                            axis=X,
                        )

                if m in DSET:
                    blk = 13 + state["nd"]
                    state["nd"] += 1
                    if halves:
                        nc.vector.reduce_max(
                            maxall[:, 108:112],
                            t[:, 1024:2048].rearrange("p (g s) -> p g s", s=S),
                            axis=X,
                        )
                    else:
                        nc.vector.reduce_max(
                            maxall[:, 8 * blk : 8 * blk + 8],
                            t[:, :].rearrange("p (g s) -> p g s", s=S),
                            axis=X,
                        )
                    return
                bi, slot = bat_of[m]
                L = len(BATCHES[bi])
                if slot == 0:
                    state["arena"] = arp.tile(
                        [128, 8192], F16, tag="arena", name=f"a{m}"
                    )
                arena = state["arena"]
                nc.scalar.copy(
                    arena[:, 2048 * slot : 2048 * (slot + 1)], t[:, :]
                )
                if slot == L - 1:
                    # f16 tensor_max halving tree over the batch (2x_1p)
                    gv = arena[:, 0 : 2048 * L].rearrange(
                        "p (g s) -> p g s", s=256
                    )
                    t1 = trp.tile([128, 4096], F16, tag="t1")
                    t1v = t1[:, 0 : 1024 * L].rearrange("p (g s) -> p g s", s=128)
                    nc.vector.tensor_max(t1v, gv[:, :, 0:128], gv[:, :, 128:256])
                    t2 = trp.tile([128, 2048], F16, tag="t2")
                    t2v = t2[:, 0 : 512 * L].rearrange("p (g s) -> p g s", s=64)
                    nc.vector.tensor_max(t2v, t1v[:, :, 0:64], t1v[:, :, 64:128])
                    t3 = trp.tile([128, 1024], F16, tag="t3")
                    t3v = t3[:, 0 : 256 * L].rearrange("p (g s) -> p g s", s=32)
                    nc.vector.tensor_max(t3v, t2v[:, :, 0:32], t2v[:, :, 32:64])
                    t4 = trp.tile([128, 512], F16, tag="t4")
                    t4v = t4[:, 0 : 128 * L].rearrange("p (g s) -> p g s", s=16)
                    nc.vector.tensor_max(t4v, t3v[:, :, 0:16], t3v[:, :, 16:32])
                    nc.vector.reduce_max(
                        maxall[:, colofs[bi] : colofs[bi] + 8 * L],
                        t4v,
                        axis=X,
                    )

            for m in range(10):
                emit_unit(m)

            # Pairwise neg term: 8 small matmuls (M=32) col-packed 4-way via
            # tile_position into ONE (128, 512) region; a single segmented
            # reduce writes maxall[:, 128:130] (local b at partitions
            # 32*(b%4) + n, column 128 + b//4).
            pt = ps.tile([128, 2048], F32, tag="chunk", name="ptw")
            for b in range(CL):
                g, j = divmod(b, 4)
                nc.tensor.matmul(
                    pt[32 * j : 32 * (j + 1), 256 * g : 256 * (g + 1)],
                    qps[:, 32 * b : 32 * (b + 1)],
                    ns[:, 256 * b : 256 * (b + 1)],
                    start=True,
                    stop=True,
                    tile_position=(0, 32 * j),
                )
            nc.vector.reduce_max(
                maxall[:, 128:130],
                pt[:, 0:512].rearrange("p (g s) -> p g s", s=S),
                axis=X,
            )

            for m in (10, 11, 15, 12, 13, 14):
                emit_unit(m)

            # n-sum via block-ones matmul: out[j, col] = sum_n maxall[32j+n, col]
            # Split at col 64 so the first half overlaps the remaining work.
            for c0, c1 in ((0, 64), (64, 130)):
                ot = ps.tile([128, 2048], F32, tag="chunk")
                nc.tensor.matmul(
                    ot[0:4, 0 : c1 - c0],
                    onesb[:, :],
                    maxall[:, c0:c1],
                    start=True,
                    stop=True,
                )
                nc.vector.tensor_copy(outsb[:, c0:c1], ot[0:4, 0 : c1 - c0])
                nc.sync.dma_start(out=out_d[:, c0:c1], in_=outsb[:, c0:c1])

    nc.finalize()
    return nc


LAST_RESULT = None


def kernel(query_embeddings, doc_embeddings, neg_doc_embeddings):
    global LAST_RESULT
    _install_ntff_shim()

    q = np.asarray(query_embeddings, dtype=np.float32)
    d = np.asarray(doc_embeddings, dtype=np.float32)
    g = np.asarray(neg_doc_embeddings, dtype=np.float32)
    assert q.shape == (B, N, D) and d.shape == (B, S, D) and g.shape == (B, S, D)

    # d-major layouts
    qT_all = np.ascontiguousarray(q.transpose(2, 0, 1).reshape(D, BN).astype(np.float16))
    ones_blk = np.zeros((D, 4), dtype=np.float16)
    ones_blk[np.arange(D), np.arange(D) // 32] = 1.0

    in_maps = []
    for k in range(NC):
        dT_k = np.ascontiguousarray(
            d[CL * k : CL * (k + 1)].transpose(2, 0, 1).reshape(D, DCOLS).astype(np.float16)
        )
        nT_k = np.ascontiguousarray(
            g[CL * k : CL * (k + 1)].transpose(2, 0, 1).reshape(D, DCOLS).astype(np.float16)
        )
        qp_k = np.ascontiguousarray(qT_all[:, CL * N * k : CL * N * (k + 1)])
        in_maps.append(
            {"qT": qT_all, "dT": dT_k, "nT": nT_k, "qp": qp_k, "ones": ones_blk}
        )

    if "nc" not in _CACHE:
        _CACHE["nc"] = _build()
    res = run_bass_kernel_spmd(_CACHE["nc"], in_maps, core_ids=list(range(NC)))
    LAST_RESULT = res

    # Assemble: scores (64, 64) and pairwise neg scores (64,)
    scores = np.empty((B, B), dtype=np.float32)
    negpair = np.empty((B,), dtype=np.float32)
    # unit m (query chunk) -> maxall col block: A units 0,2,3,4,6,7,8,10,11,
    # 12,14,15 get blocks 0..11 (tree order); D units 1,5,9,13 get 12..15.
    blk = {}
    _a = [m for m in range(16) if m not in (0, 8, 15)]
    for i, m in enumerate(_a):
        blk[m] = i
    for i, m in enumerate((0, 8, 15)):
        blk[m] = 13 + i
    for k in range(NC):
        o = res.results[k]["out"]  # (4, 130)
        for m in range(16):
            scores[4 * m : 4 * m + 4, CL * k : CL * (k + 1)] = o[
                :, 8 * blk[m] : 8 * blk[m] + 8
            ]
        for gcol in range(2):
            for j in range(4):
                negpair[CL * k + 4 * gcol + j] = o[j, 128 + gcol]

    pos = np.diagonal(scores).astype(np.float64)
    l1 = np.logaddexp(0.0, negpair.astype(np.float64) - pos).mean()
    neg_ib = (
        scores.astype(np.float64) - np.eye(B, dtype=np.float64) * NEG_INF_DIAG
    ).max(axis=1)
    l2 = np.logaddexp(0.0, neg_ib - pos).mean()
    return np.asarray((l1 + l2) / 2.0, dtype=np.float32)


# revision 33
# speedup vs baseline: 1.0309x; 1.0309x over previous
"""ColBERT pairwise + in-batch negative CE loss on 8 Trainium2 NeuronCores.

Problem shapes (hardcoded): B=64, N=32, S=256, D=128, fp32.

reference:
    pos_scores[b]  = sum_n max_s  q[b,n,:] . d[b,s,:]
    neg_scores[b]  = sum_n max_s  q[b,n,:] . neg[b,s,:]
    scores[b,c]    = sum_n max_s  q[b,n,:] . d[c,s,:]
    loss = (mean softplus(neg_scores - pos_scores)
            + mean softplus(max_offdiag_c scores[b,c] - scores[b,b])) / 2

Sharding: in-batch scores sharded over the doc dim c (8 docs per core; every
core sees all 64*32 query rows).  The pairwise-neg term is data-parallel over
b.  All operands are host-pre-transposed to d-major so the contraction dim
d=128 maps onto the PE partition dim with zero device transposes.

The kernel's true bottleneck is the PSUM drain: every score element must
leave PSUM through the vector OR scalar engine at 1 elem/cycle/lane
(HW-measured; dual-PSUM-operand reads are illegal, tensor_reduce has no 2x
mode even for 16-bit, gpsimd has no max ops and no PSUM port, DMA cannot
read PSUM).  The 16 score units (each [128 q, 8 docs x 256 s] fp32 in PSUM,
double-buffered) are therefore split between the two engines so both drain
concurrently:

- D-units {0, 4, 8, 15}: vector engine reduce_max directly from PSUM
  (~2.3us per unit).  Unit 0's reduce is split in halves emitted between
  its matmuls so the first reduce fires after 2 of 4 MMs (shorter ramp);
  unit 15 is direct so the tail ends with a cheap reduce, not a tree.
- A-units (the other 12): scalar engine copies PSUM -> SBUF f16 (~2.0us,
  1 elem/cycle), and the vector engine finishes with a tensor_max halving
  tree, which runs at 2 results/cycle in 16-bit (2x_1p, consuming 4
  elem/cycle) and is batched over 3 units to amortize per-op overhead
  (~1.4us per unit).  maxall columns are assigned in drain order and
  remapped on the host.

Per core the device produces a (4, 130) fp32 tile:
  cols 0..127:  col 8*blk+c, row j -> sum_n max_s (q[. ] . d_local[c])
                (blk = drain-order block of query chunk m; host remaps)
  cols 128/129: col 128+g, row j  ->  neg_scores for local b = 4g+j
The host assembles the full (64, 64) scores matrix + the 64 neg pairwise
scores and applies the trivial softplus/mean epilogue (128 scalars).
"""

import sys

import numpy as np


def _ensure_path():
    try:
        import concourse  # noqa: F401
    except ImportError:
        sys.path.insert(0, "/opt/trn_rl_repo")


_ensure_path()

import concourse.bacc as bacc  # noqa: E402
import concourse.mybir as mybir  # noqa: E402
from concourse.bass_utils import run_bass_kernel_spmd  # noqa: E402
from concourse.tile import TileContext  # noqa: E402

B, N, S, D = 64, 32, 256, 128
NC = 8
CL = B // NC  # docs / queries per core (8)
BN = B * N  # 2048 query rows
DCOLS = CL * S  # 2048 doc columns per core
NEG_INF_DIAG = 1000000.0

F32 = mybir.dt.float32
F16 = mybir.dt.float16
MMDT = mybir.dt.float16  # dtype used by the matmul operands

_CACHE = {}


def _install_ntff_shim():
    """Best-effort: register the axon NTFF profile hook so BASS_TRACE=1
    produces hardware profiles.  Safe no-op when unavailable."""
    try:
        import types

        import antenv

        if "antenv.axon_hooks" in sys.modules:
            return
        import trn_agent_boot.trn_boot as tb

        mod = types.ModuleType("antenv.axon_hooks")
        _hook = [None]
        mod.set_axon_ntff_profile_hook = lambda h: _hook.__setitem__(0, h)
        mod.get_axon_ntff_profile_hook = lambda: _hook[0]
        sys.modules["antenv.axon_hooks"] = mod
        antenv.axon_hooks = mod
        mod.set_axon_ntff_profile_hook(
            tb._ntff_profile_via_ctypes("/opt/axon/libaxon_pjrt.so")
        )
    except Exception:
        pass


def _build():
    nc = bacc.Bacc("TRN2", target_bir_lowering=False, debug=False, num_devices=NC)
    qT = nc.dram_tensor("qT", [D, BN], MMDT, kind="ExternalInput")
    dT = nc.dram_tensor("dT", [D, DCOLS], MMDT, kind="ExternalInput")
    nT = nc.dram_tensor("nT", [D, DCOLS], MMDT, kind="ExternalInput")
    qp = nc.dram_tensor("qp", [D, CL * N], MMDT, kind="ExternalInput")
    ones = nc.dram_tensor("ones", [D, 4], F16, kind="ExternalInput")
    out_d = nc.dram_tensor("out", [4, 130], F32, kind="ExternalOutput")

    X = mybir.AxisListType.X

    with TileContext(nc) as tc:
        with (
            tc.tile_pool(name="sb", bufs=1) as sb,
            tc.tile_pool(name="ar", bufs=3) as arp,
            tc.tile_pool(name="tr", bufs=2) as trp,
            tc.tile_pool(name="ps", bufs=2, space="PSUM") as ps,
        ):
            qs = sb.tile([D, BN], MMDT, tag="qs")
            ds = sb.tile([D, DCOLS], MMDT, tag="ds")
            ns = sb.tile([D, DCOLS], MMDT, tag="ns")
            qps = sb.tile([D, CL * N], MMDT, tag="qps")
            onesb = sb.tile([D, 4], F16, tag="ones")
            maxall = sb.tile([128, 130], F16, tag="maxall")
            outsb = sb.tile([4, 130], F32, tag="outsb")

            # Input DMAs spread over four hardware queues (each engine can
            # issue DMAs) so ds+first q chunk land as early as possible.
            nc.sync.dma_start(out=qs[:, 0:128], in_=qT[:, 0:128])
            for p8 in range(8):
                eng = nc.sync if p8 % 2 == 0 else nc.scalar
                sl = slice(256 * p8, 256 * (p8 + 1))
                eng.dma_start(out=ds[:, sl], in_=dT[:, sl])
            nc.sync.dma_start(out=qs[:, 128:1024], in_=qT[:, 128:1024])
            nc.sync.dma_start(out=qs[:, 1024:2048], in_=qT[:, 1024:2048])
            for p, eng in zip(range(4), (nc.sync, nc.sync, nc.sync, nc.sync)):
                sl = slice(512 * p, 512 * (p + 1))
                eng.dma_start(out=ns[:, sl], in_=nT[:, sl])
            nc.sync.dma_start(out=qps[:, :], in_=qp[:, :])
            nc.sync.dma_start(out=onesb[:, :], in_=ones[:, :])

            # In-batch term: query chunk m (128 rows) x all 2048 local doc
            # cols -> PSUM [128, 2048].  D units {0,4,8,15} are reduced
            # directly by the vector engine (maxall col blocks 12..15); the
            # other 12 are drained by the scalar engine into an f16 arena
            # and max-tree'd by the vector engine in batches of 3 (col
            # blocks 0..11 in batch order; host remaps).  The pairwise term
            # is emitted mid-stream so its reduce stays off the tail, and
            # unit 15 is emitted before units 12-14 so the final tail is a
            # tree only.
            # Pre-warm the PE's HAM clock gate with dummy matmuls on
            # memset data (no DMA dependency) so the real matmuls run at
            # 2.4GHz from the start instead of paying ~3.4us of 1.2GHz ramp.
            wa = sb.tile([D, 128], F16, tag="wa")
            wb = sb.tile([D, 512], F16, tag="wb")
            nc.gpsimd.memset(wa[:, :], 0.0)
            nc.gpsimd.memset(wb[:, :], 0.0)
            wt = ps.tile([128, 2048], F32, tag="chunk", name="warm")
            for w in range(4):
                nc.tensor.matmul(
                    wt[:, 512 * (w % 4) : 512 * (w % 4 + 1)],
                    wa[:, :],
                    wb[:, :],
                    start=True,
                    stop=True,
                )

            DSET = {0, 4, 8, 15}
            # A-unit batches for the f16 max-trees; the last batch is small
            # so the tail tree after the final ACT copy is short.
            BATCHES = [[1, 2, 3], [5, 6, 7], [9, 10, 11], [12, 13, 14]]
            bat_of = {}
            for _bi, _bb in enumerate(BATCHES):
                for _sl, _mm in enumerate(_bb):
                    bat_of[_mm] = (_bi, _sl)
            colofs = [0]
            for _bb in BATCHES:
                colofs.append(colofs[-1] + 8 * len(_bb))
            state = {"arena": None, "nd": 0}

            def emit_unit(m):
                t = ps.tile([128, 2048], F32, tag="chunk", name=f"u{m}")
                halves = (m == 0)
                for u in range(4):
                    nc.tensor.matmul(
                        t[:, 512 * u : 512 * (u + 1)],
                        qs[:, 128 * m : 128 * (m + 1)],
                        ds[:, 512 * u : 512 * (u + 1)],
                        start=True,
                        stop=True,
                    )
                    if halves and u == 1:
                        nc.vector.reduce_max(
                            maxall[:, 96:100],
                            t[:, 0:1024].rearrange("p (g s) -> p g s", s=S),
                            axis=X,
                        )

                if m in DSET:
                    blk = 12 + state["nd"]
                    state["nd"] += 1
                    if halves:
                        nc.vector.reduce_max(
                            maxall[:, 100:104],
                            t[:, 1024:2048].rearrange("p (g s) -> p g s", s=S),
                            axis=X,
                        )
                    else:
                        nc.vector.reduce_max(
                            maxall[:, 8 * blk : 8 * blk + 8],
                            t[:, :].rearrange("p (g s) -> p g s", s=S),
                            axis=X,
                        )
                    return
                bi, slot = bat_of[m]
                L = len(BATCHES[bi])
                if slot == 0:
                    state["arena"] = arp.tile(
                        [128, 8192], F16, tag="arena", name=f"a{m}"
                    )
                arena = state["arena"]
                nc.scalar.copy(
                    arena[:, 2048 * slot : 2048 * (slot + 1)], t[:, :]
                )
                if slot == L - 1:
                    # f16 tensor_max halving tree over the batch (2x_1p)
                    gv = arena[:, 0 : 2048 * L].rearrange(
                        "p (g s) -> p g s", s=256
                    )
                    t1 = trp.tile([128, 4096], F16, tag="t1")
                    t1v = t1[:, 0 : 1024 * L].rearrange("p (g s) -> p g s", s=128)
                    nc.vector.tensor_max(t1v, gv[:, :, 0:128], gv[:, :, 128:256])
                    t2 = trp.tile([128, 2048], F16, tag="t2")
                    t2v = t2[:, 0 : 512 * L].rearrange("p (g s) -> p g s", s=64)
                    nc.vector.tensor_max(t2v, t1v[:, :, 0:64], t1v[:, :, 64:128])
                    t3 = trp.tile([128, 1024], F16, tag="t3")
                    t3v = t3[:, 0 : 256 * L].rearrange("p (g s) -> p g s", s=32)
                    nc.vector.tensor_max(t3v, t2v[:, :, 0:32], t2v[:, :, 32:64])
                    t4 = trp.tile([128, 512], F16, tag="t4")
                    t4v = t4[:, 0 : 128 * L].rearrange("p (g s) -> p g s", s=16)
                    nc.vector.tensor_max(t4v, t3v[:, :, 0:16], t3v[:, :, 16:32])
                    nc.vector.reduce_max(
                        maxall[:, colofs[bi] : colofs[bi] + 8 * L],
                        t4v,
                        axis=X,
                    )

            for m in range(10):
                emit_unit(m)

            # Pairwise neg term: 8 small matmuls (M=32) col-packed 4-way via
            # tile_position into ONE (128, 512) region; a single segmented
            # reduce writes maxall[:, 128:130] (local b at partitions
            # 32*(b%4) + n, column 128 + b//4).
            pt = ps.tile([128, 2048], F32, tag="chunk", name="ptw")
            for b in range(CL):
                g, j = divmod(b, 4)
                nc.tensor.matmul(
                    pt[32 * j : 32 * (j + 1), 256 * g : 256 * (g + 1)],
                    qps[:, 32 * b : 32 * (b + 1)],
                    ns[:, 256 * b : 256 * (b + 1)],
                    start=True,
                    stop=True,
                    tile_position=(0, 32 * j),
                )
            nc.vector.reduce_max(
                maxall[:, 128:130],
                pt[:, 0:512].rearrange("p (g s) -> p g s", s=S),
                axis=X,
            )

            for m in (10, 11, 15, 12, 13, 14):
                emit_unit(m)

            # n-sum via block-ones matmul: out[j, col] = sum_n maxall[32j+n, col]
            # Split at col 64 so the first half overlaps the remaining work.
            for c0, c1 in ((0, 64), (64, 130)):
                ot = ps.tile([128, 2048], F32, tag="chunk")
                nc.tensor.matmul(
                    ot[0:4, 0 : c1 - c0],
                    onesb[:, :],
                    maxall[:, c0:c1],
                    start=True,
                    stop=True,
                )
                nc.vector.tensor_copy(outsb[:, c0:c1], ot[0:4, 0 : c1 - c0])
                nc.sync.dma_start(out=out_d[:, c0:c1], in_=outsb[:, c0:c1])

    nc.finalize()
    return nc


LAST_RESULT = None


def kernel(query_embeddings, doc_embeddings, neg_doc_embeddings):
    global LAST_RESULT
    _install_ntff_shim()

    q = np.asarray(query_embeddings, dtype=np.float32)
    d = np.asarray(doc_embeddings, dtype=np.float32)
    g = np.asarray(neg_doc_embeddings, dtype=np.float32)
    assert q.shape == (B, N, D) and d.shape == (B, S, D) and g.shape == (B, S, D)

    # d-major layouts
    qT_all = np.ascontiguousarray(q.transpose(2, 0, 1).reshape(D, BN).astype(np.float16))
    ones_blk = np.zeros((D, 4), dtype=np.float16)
    ones_blk[np.arange(D), np.arange(D) // 32] = 1.0

    in_maps = []
    for k in range(NC):
        dT_k = np.ascontiguousarray(
            d[CL * k : CL * (k + 1)].transpose(2, 0, 1).reshape(D, DCOLS).astype(np.float16)
        )
        nT_k = np.ascontiguousarray(
            g[CL * k : CL * (k + 1)].transpose(2, 0, 1).reshape(D, DCOLS).astype(np.float16)
        )
        qp_k = np.ascontiguousarray(qT_all[:, CL * N * k : CL * N * (k + 1)])
        in_maps.append(
            {"qT": qT_all, "dT": dT_k, "nT": nT_k, "qp": qp_k, "ones": ones_blk}
        )

    if "nc" not in _CACHE:
        _CACHE["nc"] = _build()
    res = run_bass_kernel_spmd(_CACHE["nc"], in_maps, core_ids=list(range(NC)))
    LAST_RESULT = res

    # Assemble: scores (64, 64) and pairwise neg scores (64,)
    scores = np.empty((B, B), dtype=np.float32)
    negpair = np.empty((B,), dtype=np.float32)
    # unit m (query chunk) -> maxall col block: A units 0,2,3,4,6,7,8,10,11,
    # 12,14,15 get blocks 0..11 (tree order); D units 1,5,9,13 get 12..15.
    blk = {}
    _a = [m for m in range(16) if m not in (0, 4, 8, 15)]
    for i, m in enumerate(_a):
        blk[m] = i
    for i, m in enumerate((0, 4, 8, 15)):
        blk[m] = 12 + i
    for k in range(NC):
        o = res.results[k]["out"]  # (4, 130)
        for m in range(16):
            scores[4 * m : 4 * m + 4, CL * k : CL * (k + 1)] = o[
                :, 8 * blk[m] : 8 * blk[m] + 8
            ]
        for gcol in range(2):
            for j in range(4):
                negpair[CL * k + 4 * gcol + j] = o[j, 128 + gcol]

    pos = np.diagonal(scores).astype(np.float64)
    l1 = np.logaddexp(0.0, negpair.astype(np.float64) - pos).mean()
    neg_ib = (
        scores.astype(np.float64) - np.eye(B, dtype=np.float64) * NEG_INF_DIAG
    ).max(axis=1)
    l2 = np.logaddexp(0.0, neg_ib - pos).mean()
    return np.asarray((l1 + l2) / 2.0, dtype=np.float32)
